# revision 2
# baseline (speedup 1.0000x reference)
"""AlphaPortfolio model distributed across 8 Trainium2 NeuronCores.

Sharding (per the hint): the sequence axis N = B*A is sharded 8 ways for the
SREM encoder (assets are independent through the transformer) — each core
runs 512 of the 4096 sequences through in-proj, self-attention and the heavy
32->2048->32 FFN. The CAAN attention needs the full asset axis for its
keys/values, so asset representations are gathered before the CAAN stage,
whose queries are sharded on the asset axis. The per-batch top/bottom-G
selection is O(B*A) scalar bookkeeping on the (4,1024) winner vector and
runs on host with exactly the reference's stable double-argsort semantics.

The model is compiled as several small XLA stages (one per encoder
sub-block) rather than one monolithic graph — the monolithic graph blows
the on-chip state-buffer allocator in neuronx-cc. Each stage falls back to
CPU transparently if its device compile fails, so the kernel always
produces correct output.
"""

import numpy as np
import jax
import jax.numpy as jnp
from jax.sharding import Mesh, NamedSharding, PartitionSpec as P

D_MODEL = 32
NHEAD = 4
HEAD = D_MODEL // NHEAD
DFF = 2048
NLAYERS = 2
D_ATTN = 16
G = 20
EPS = 1e-5
NEG = -1e9

N_CORES = 8
_INV_SQRT_HEAD = np.float32(1.0 / np.sqrt(HEAD))
_INV_SQRT_DATTN = np.float32(1.0 / np.sqrt(D_ATTN))


def _ln(x, g, b):
    m = jnp.mean(x, -1, keepdims=True)
    v = jnp.mean((x - m) ** 2, -1, keepdims=True)
    return (x - m) / jnp.sqrt(v + EPS) * g + b


# ---------------- stages (each becomes one small NEFF) ----------------

def _in_proj(x2, w_in, b_in):
    # x2: (N, L, F) -> (N, L, d)
    return x2 @ w_in + b_in


def _attn_layer(h, Wqkv_i, bqkv_i, Wo_i, bo_i, g1, b1):
    N, L, d = h.shape
    qkv = h @ Wqkv_i + bqkv_i
    q, k, v = jnp.split(qkv, 3, axis=-1)
    q = q.reshape(N, L, NHEAD, HEAD).transpose(0, 2, 1, 3)  # (N,H,L,hd)
    k = k.reshape(N, L, NHEAD, HEAD).transpose(0, 2, 1, 3)
    v = v.reshape(N, L, NHEAD, HEAD).transpose(0, 2, 1, 3)
    att = jnp.einsum('nhqd,nhkd->nhqk', q, k) * _INV_SQRT_HEAD
    att = jax.nn.softmax(att, axis=-1)
    o = jnp.einsum('nhqk,nhkd->nhqd', att, v)
    o = o.transpose(0, 2, 1, 3).reshape(N, L, D_MODEL)
    return _ln(h + o @ Wo_i + bo_i, g1, b1)


def _ffn_layer(h, W1_i, b1_i, W2_i, b2_i, g2, b2):
    ffn = jax.nn.relu(h @ W1_i + b1_i) @ W2_i + b2_i
    return _ln(h + ffn, g2, b2)


def _caan(ar, mask, Wq, bq, Wk, bk, Wv, bv, Ws, bs):
    # ar: (B, A, L*d)
    Q = ar @ Wq + bq
    K = ar @ Wk + bk
    V = ar @ Wv + bv
    scores = jnp.einsum('bqd,bkd->bqk', Q, K) * _INV_SQRT_DATTN
    maskf = mask.astype(scores.dtype)
    scores = scores + (1.0 - maskf)[:, None, :] * NEG
    attn = jax.nn.softmax(scores, axis=-1)
    attn_vec = jnp.einsum('bqk,bkd->bqd', attn, V)
    ws = jnp.tanh(attn_vec @ Ws + bs)[..., 0]
    return jnp.where(mask, ws, NEG)


# ---------------- host tail: exact reference selection semantics ----------------

def _select_host(winner, mask):
    B, A = winner.shape
    n_valid = mask.sum(-1)
    g_adj = np.where(n_valid >= 2, np.minimum(G, n_valid // 2), 1).astype(np.int64)
    rank_long = np.argsort(np.argsort(-winner, axis=-1, kind='stable'),
                           axis=-1, kind='stable')
    short_key = np.where(mask, winner, np.float32(-NEG))
    rank_short = np.argsort(np.argsort(short_key, axis=-1, kind='stable'),
                            axis=-1, kind='stable')
    is_long = (rank_long < g_adj[:, None]) & mask
    is_short = (rank_short < g_adj[:, None]) & mask

    def softmax(z):
        z = z - z.max(-1, keepdims=True)
        e = np.exp(z, dtype=np.float32)
        return e / e.sum(-1, keepdims=True)

    w_long = softmax(np.where(is_long, winner, np.float32(NEG)))
    w_short = softmax(np.where(is_short, -winner, np.float32(NEG)))
    pw = np.where(is_short, -w_short, np.where(is_long, w_long, np.float32(0.0)))
    pw = np.where((n_valid > 0)[:, None], pw, np.float32(0.0))
    return pw.astype(np.float32)


# ---------------- stage runner with per-stage CPU fallback ----------------

_mesh = None
_dev_jits = {}
_cpu_jits = {}
_stage_broken = set()


def _get_mesh():
    global _mesh
    if _mesh is None:
        _mesh = Mesh(np.array(jax.devices()[:N_CORES]), ('x',))
    return _mesh


_STAGES = {
    'in_proj': _in_proj,
    'attn0': _attn_layer, 'attn1': _attn_layer,
    'ffn0': _ffn_layer, 'ffn1': _ffn_layer,
    'caan': _caan,
}


def _run_stage(name, out_spec, arg_specs, *args):
    """Run stage on the trn2 mesh; fall back to CPU if device compile fails."""
    fn = _STAGES[name]
    mesh = _get_mesh()
    if name not in _stage_broken:
        try:
            if name not in _dev_jits:
                _dev_jits[name] = jax.jit(
                    fn,
                    in_shardings=tuple(NamedSharding(mesh, s) for s in arg_specs),
                    out_shardings=NamedSharding(mesh, out_spec),
                )
            dev_args = [
                jax.device_put(a, NamedSharding(mesh, s))
                for a, s in zip(args, arg_specs)
            ]
            out = _dev_jits[name](*dev_args)
            out.block_until_ready()
            return out
        except Exception as e:  # compile or runtime failure -> CPU fallback
            import sys
            print(f'[kernel] stage {name} fell back to CPU: {type(e).__name__}',
                  file=sys.stderr)
            _stage_broken.add(name)
    if name not in _cpu_jits:
        cpu = jax.devices('cpu')[0]
        _cpu_jits[name] = jax.jit(fn, device=cpu)
    host_args = [np.asarray(a) for a in args]
    return _cpu_jits[name](*host_args)


def kernel(**inputs):
    x = np.asarray(inputs['x'], np.float32)
    mask = np.asarray(inputs['mask'], bool)
    B, A, L, F = x.shape
    N = B * A
    gf = lambda k: np.asarray(inputs[k], np.float32)
    w_in, b_in = gf('w_in'), gf('b_in')
    Wqkv, bqkv, Wo, bo = gf('Wqkv'), gf('bqkv'), gf('Wo'), gf('bo')
    ln1_g, ln1_b = gf('ln1_g'), gf('ln1_b')
    W1, b1, W2, b2 = gf('W1'), gf('b1'), gf('W2'), gf('b2')
    ln2_g, ln2_b = gf('ln2_g'), gf('ln2_b')
    Wq, bq, Wk, bk = gf('Wq'), gf('bq'), gf('Wk'), gf('bk')
    Wv, bv, Ws, bs = gf('Wv'), gf('bv'), gf('Ws'), gf('bs')

    seq = P('x', None, None)   # (N, L, *) sharded over sequences
    rep3 = P(None, None)
    rep1 = P(None)

    h = _run_stage('in_proj', seq, (seq, rep3, rep1), x.reshape(N, L, F), w_in, b_in)
    for i in range(NLAYERS):
        h = _run_stage(f'attn{i}', seq,
                       (seq, rep3, rep1, rep3, rep1, rep1, rep1),
                       h, Wqkv[i], bqkv[i], Wo[i], bo[i], ln1_g[i], ln1_b[i])
        h = _run_stage(f'ffn{i}', seq,
                       (seq, rep3, rep1, rep3, rep1, rep1, rep1),
                       h, W1[i], b1[i], W2[i], b2[i], ln2_g[i], ln2_b[i])

    ar = np.asarray(h, np.float32).reshape(B, A, L * D_MODEL)
    winner = _run_stage('caan', P(None, None),
                        (P(None, 'x', None), P(None, None),
                         rep3, rep1, rep3, rep1, rep3, rep1, rep3, rep1),
                        ar, mask, Wq, bq, Wk, bk, Wv, bv, Ws, bs)
    winner = np.asarray(winner, np.float32)

    pw = _select_host(winner, mask)
    return pw, winner


# revision 6
# speedup vs baseline: 7.9752x; 7.9752x over previous
"""AlphaPortfolio model distributed across 8 Trainium2 NeuronCores.

Sharding (per the hint): the sequence axis N = B*A is sharded 8 ways for the
SREM encoder (assets are independent through the transformer) — each core
runs 512 of the 4096 sequences through in-proj, self-attention and the heavy
32->2048->32 FFN. The CAAN attention needs the full asset axis for its
keys/values, so asset representations are gathered before the CAAN stage,
whose queries are sharded on the asset axis. The per-batch top/bottom-G
selection is O(B*A) scalar bookkeeping on the (4,1024) winner vector and
runs on host with exactly the reference's stable double-argsort semantics.

The model is compiled as several small XLA stages (one per encoder
sub-block) rather than one monolithic graph — the monolithic graph blows
the on-chip state-buffer allocator in neuronx-cc. Each stage falls back to
CPU transparently if its device compile fails, so the kernel always
produces correct output.
"""

import numpy as np
import jax
import jax.numpy as jnp
from jax.sharding import Mesh, NamedSharding, PartitionSpec as P

D_MODEL = 32
NHEAD = 4
HEAD = D_MODEL // NHEAD
DFF = 2048
NLAYERS = 2
D_ATTN = 16
G = 20
EPS = 1e-5
NEG = -1e9

N_CORES = 8
_INV_SQRT_HEAD = np.float32(1.0 / np.sqrt(HEAD))
_INV_SQRT_DATTN = np.float32(1.0 / np.sqrt(D_ATTN))


def _ln(x, g, b):
    m = jnp.mean(x, -1, keepdims=True)
    v = jnp.mean((x - m) ** 2, -1, keepdims=True)
    return (x - m) / jnp.sqrt(v + EPS) * g + b


# ---------------- stages (each becomes one small NEFF) ----------------

def _in_proj(x2, w_in, b_in):
    # x2: (N, L, F) -> (N, L, d)
    return x2 @ w_in + b_in


def _attn_layer(h, Wqkv_i, bqkv_i, Wo_i, bo_i, g1, b1):
    N, L, d = h.shape
    qkv = h @ Wqkv_i + bqkv_i
    q, k, v = jnp.split(qkv, 3, axis=-1)
    q = q.reshape(N, L, NHEAD, HEAD).transpose(0, 2, 1, 3)  # (N,H,L,hd)
    k = k.reshape(N, L, NHEAD, HEAD).transpose(0, 2, 1, 3)
    v = v.reshape(N, L, NHEAD, HEAD).transpose(0, 2, 1, 3)
    att = jnp.einsum('nhqd,nhkd->nhqk', q, k) * _INV_SQRT_HEAD
    att = jax.nn.softmax(att, axis=-1)
    o = jnp.einsum('nhqk,nhkd->nhqd', att, v)
    o = o.transpose(0, 2, 1, 3).reshape(N, L, D_MODEL)
    return _ln(h + o @ Wo_i + bo_i, g1, b1)


def _ffn_layer(h, W1_i, b1_i, W2_i, b2_i, g2, b2):
    ffn = jax.nn.relu(h @ W1_i + b1_i) @ W2_i + b2_i
    return _ln(h + ffn, g2, b2)


def _caan(h, mask, Wq, bq, Wk, bk, Wv, bv, Ws, bs):
    # h: (N, L, d) -> asset_repr (B, A, L*d) on device
    B, A = mask.shape
    ar = h.reshape(B, A, h.shape[1] * h.shape[2])
    Q = ar @ Wq + bq
    K = ar @ Wk + bk
    V = ar @ Wv + bv
    scores = jnp.einsum('bqd,bkd->bqk', Q, K) * _INV_SQRT_DATTN
    maskf = mask.astype(scores.dtype)
    scores = scores + (1.0 - maskf)[:, None, :] * NEG
    attn = jax.nn.softmax(scores, axis=-1)
    attn_vec = jnp.einsum('bqk,bkd->bqd', attn, V)
    ws = jnp.tanh(attn_vec @ Ws + bs)[..., 0]
    return jnp.where(mask, ws, NEG)


# ---------------- host tail: exact reference selection semantics ----------------

def _select_host(winner, mask):
    B, A = winner.shape
    n_valid = mask.sum(-1)
    g_adj = np.where(n_valid >= 2, np.minimum(G, n_valid // 2), 1).astype(np.int64)
    rank_long = np.argsort(np.argsort(-winner, axis=-1, kind='stable'),
                           axis=-1, kind='stable')
    short_key = np.where(mask, winner, np.float32(-NEG))
    rank_short = np.argsort(np.argsort(short_key, axis=-1, kind='stable'),
                            axis=-1, kind='stable')
    is_long = (rank_long < g_adj[:, None]) & mask
    is_short = (rank_short < g_adj[:, None]) & mask

    def softmax(z):
        z = z - z.max(-1, keepdims=True)
        e = np.exp(z, dtype=np.float32)
        return e / e.sum(-1, keepdims=True)

    w_long = softmax(np.where(is_long, winner, np.float32(NEG)))
    w_short = softmax(np.where(is_short, -winner, np.float32(NEG)))
    pw = np.where(is_short, -w_short, np.where(is_long, w_long, np.float32(0.0)))
    pw = np.where((n_valid > 0)[:, None], pw, np.float32(0.0))
    return pw.astype(np.float32)


# ---------------- stage runner with per-stage CPU fallback ----------------

_mesh = None
_dev_jits = {}
_cpu_jits = {}
_stage_broken = set()
_stage_warm = set()
_put_cache = {}


def _cached_put(a, sharding):
    """device_put with a content-keyed cache (weights repeat across calls)."""
    if not isinstance(a, np.ndarray) or a.nbytes > 4 << 20:
        return jax.device_put(a, sharding)
    key = (hash(a.tobytes()), a.shape, str(a.dtype), str(sharding))
    hit = _put_cache.get(key)
    if hit is None:
        hit = _put_cache[key] = jax.device_put(a, sharding)
    return hit


def _get_mesh():
    global _mesh
    if _mesh is None:
        _mesh = Mesh(np.array(jax.devices()[:N_CORES]), ('x',))
    return _mesh


_STAGES = {
    'in_proj': _in_proj,
    'attn0': _attn_layer, 'attn1': _attn_layer,
    'ffn0': _ffn_layer, 'ffn1': _ffn_layer,
    'caan': _caan,
}


def _run_stage(name, out_spec, arg_specs, *args):
    """Run stage on the trn2 mesh; fall back to CPU if device compile fails."""
    fn = _STAGES[name]
    mesh = _get_mesh()
    if name not in _stage_broken:
        try:
            if name not in _dev_jits:
                _dev_jits[name] = jax.jit(
                    fn,
                    in_shardings=tuple(NamedSharding(mesh, s) for s in arg_specs),
                    out_shardings=NamedSharding(mesh, out_spec),
                )
            dev_args = [
                a if isinstance(a, jax.Array)
                else _cached_put(a, NamedSharding(mesh, s))
                for a, s in zip(args, arg_specs)
            ]
            out = _dev_jits[name](*dev_args)
            if name not in _stage_warm:
                # block only on first run so compile errors surface here;
                # afterwards let stages pipeline asynchronously
                out.block_until_ready()
                _stage_warm.add(name)
            return out
        except Exception as e:  # compile or runtime failure -> CPU fallback
            import sys
            print(f'[kernel] stage {name} fell back to CPU: {type(e).__name__}',
                  file=sys.stderr)
            _stage_broken.add(name)
    if name not in _cpu_jits:
        cpu = jax.devices('cpu')[0]
        _cpu_jits[name] = jax.jit(fn, device=cpu)
    host_args = [np.asarray(a) for a in args]
    return _cpu_jits[name](*host_args)


def kernel(**inputs):
    x = np.asarray(inputs['x'], np.float32)
    mask = np.asarray(inputs['mask'], bool)
    B, A, L, F = x.shape
    N = B * A
    gf = lambda k: np.asarray(inputs[k], np.float32)
    w_in, b_in = gf('w_in'), gf('b_in')
    Wqkv, bqkv, Wo, bo = gf('Wqkv'), gf('bqkv'), gf('Wo'), gf('bo')
    ln1_g, ln1_b = gf('ln1_g'), gf('ln1_b')
    W1, b1, W2, b2 = gf('W1'), gf('b1'), gf('W2'), gf('b2')
    ln2_g, ln2_b = gf('ln2_g'), gf('ln2_b')
    Wq, bq, Wk, bk = gf('Wq'), gf('bq'), gf('Wk'), gf('bk')
    Wv, bv, Ws, bs = gf('Wv'), gf('bv'), gf('Ws'), gf('bs')

    seq = P('x', None, None)   # (N, L, *) sharded over sequences
    rep3 = P(None, None)
    rep1 = P(None)

    h = _run_stage('in_proj', seq, (seq, rep3, rep1), x.reshape(N, L, F), w_in, b_in)
    for i in range(NLAYERS):
        h = _run_stage(f'attn{i}', seq,
                       (seq, rep3, rep1, rep3, rep1, rep1, rep1),
                       h, Wqkv[i], bqkv[i], Wo[i], bo[i], ln1_g[i], ln1_b[i])
        h = _run_stage(f'ffn{i}', seq,
                       (seq, rep3, rep1, rep3, rep1, rep1, rep1),
                       h, W1[i], b1[i], W2[i], b2[i], ln2_g[i], ln2_b[i])

    winner = _run_stage('caan', P(None, None),
                        (seq, P(None, None),
                         rep3, rep1, rep3, rep1, rep3, rep1, rep3, rep1),
                        h, mask, Wq, bq, Wk, bk, Wv, bv, Ws, bs)
    winner = np.asarray(winner, np.float32)

    pw = _select_host(winner, mask)
    return pw, winner
